# revision 44
# baseline (speedup 1.0000x reference)
"""CapsLayer2D Trainium2 kernel (8-core SPMD, data-parallel over batch).

Math: per position p (of B*R*C) and capsule n:
  U[n,i,o] = sum_e x[p,i,e] * W[n,i,e,o]          (u_hat)
  b0 = 1/64; 2x { v = squash(sum_i b*U); b += sum_o U*v }; out = squash(sum_i b*U)

Since b = 1/64 + A (A = accumulated agreement), sum_i b*U = s0 + sum_i A*U
with s0 = (1/64) sum_i U - so routing tracks A only (no memset, A in bf16).

Mapping:
  - 8 cores, 2 batches each -> 392 positions/core, 4 pos-blocks of 98.
  - u_hat per (block, g-chunk): one K=16 matmul per input capsule (g,i8)
    against the compact nonzero block w8 of the block-diagonal W (x is sent
    with e on partitions so every matmul sits at base partition 0); PSUM
    [98,1280] per g, Act-drained to bf16 SBUF. Writes that would cross a
    2KB PSUM bank are split (HW corrupts bank-crossing matmul stores).
    s0 comes from a dense K=128 accumulating chain (8 matmuls/block).
  - Routing iterations as elementwise mul + segmented-reduce add-trees.
  - u_hat columns are laid out (g, i8, o, n2) [NOT (..., n2, o)]: with the
    capsule-pair index n2 innermost, every operand of every big mul/tree is
    last-dim-contiguous bf16 (broadcasts over v / A land on middle dims), so
    all DVE TensorTensor ops hit the 2x perf mode.
  - The mul/tree work of capsule-pair unit f=4 runs on the Pool engine
    (gpsimd) in parallel with f=0..3 on DVE, roughly balancing the two.
  - Host pre-builds bf16 xT (e-major), xF (dense-major), w8, wd.
"""
import numpy as np

import concourse.bacc as bacc
import concourse.bass as bass
import concourse.mybir as mybir
import concourse.tile as tile
from concourse.bass_utils import run_bass_kernel_spmd

N_CORES = 8
B, R, C = 16, 14, 14
N_IN, D_IN = 64, 16          # i, e
N_CAPS, CAPS_DIM = 10, 16    # n, o
IE = N_IN * D_IN             # 1024
POS = (B // N_CORES) * R * C # 392 positions per core
BLK = 98                     # pos-block size
NBLK = POS // BLK            # 4
NF = N_CAPS // 2             # 5 units of 2 capsules
NCH = IE // 128              # 8 contraction chunks
F32 = mybir.dt.float32

# u_hat matmuls run in bf16 (1 col/cycle at any N; fp32 is 4x slower,
# fp32r needs producer-side rounding the DMA can't provide).
BF16 = mybir.dt.bfloat16

# DVE handles capsule-pair units f < FD, Pool (gpsimd) handles f >= FD.
FD = 4


def _squash_on(nc, pool, s_t, v_ap, lowp=False):
    """v = squash(s) for an (f, o, n2)-ordered s tile [P, 160] (routing
    layout). v_ap must be a [P, NF, 16, 2]-shaped AP in op dims (f, o, n2).
    Square runs on DVE (cheap f32 mul) to avoid an Act round-trip; only
    Sqrt uses the Act engine.
    """
    P = s_t.shape[0]
    s_fon = s_t[:].rearrange("p (f o n) -> p f o n", o=16, n=2)
    sq = pool.tile([P, 160], F32, tag="sq")
    nc.vector.tensor_mul(sq[:], s_t[:], s_t[:])
    # reduce over o (middle in storage): 4D view [P, f, n2, o], axis=X
    q = pool.tile([P, N_CAPS], F32, tag="q")
    nc.vector.tensor_reduce(q[:].rearrange("p (f n) -> p f n", n=2),
                            sq[:].rearrange("p (f o n) -> p f n o",
                                            o=16, n=2),
                            axis=mybir.AxisListType.X, op=mybir.AluOpType.add)
    rt = pool.tile([P, N_CAPS], F32, tag="rt")
    nc.scalar.activation(rt[:], q[:], mybir.ActivationFunctionType.Sqrt)
    qp = pool.tile([P, N_CAPS], F32, tag="qp")
    nc.vector.tensor_scalar_add(qp[:], q[:], 1.0)
    rc = pool.tile([P, N_CAPS], F32, tag="rc")
    nc.vector.reciprocal(rc[:], qp[:])
    al = pool.tile([P, N_CAPS], F32, tag="al")
    nc.vector.tensor_mul(al[:], rt[:], rc[:])
    # al is [P, (f, n2)]; broadcast over o (middle dim of (f, o, n2))
    alb = al[:].rearrange("p (f n) -> p f n", n=2) \
        .unsqueeze(2).broadcast_to([P, NF, 16, 2])
    if lowp:
        with nc.allow_low_precision("bf16 v"):
            nc.vector.tensor_mul(v_ap, s_fon, alb)
    else:
        nc.vector.tensor_mul(v_ap, s_fon, alb)


def build_kernel(dbg=False, repeat=1):
    nc = bacc.Bacc("TRN2", target_bir_lowering=False, debug=False,
                   num_devices=N_CORES)
    xT = nc.dram_tensor("xT", [D_IN, N_IN * POS], BF16,
                        kind="ExternalInput").ap()
    xF = nc.dram_tensor("xF", [128, NCH * POS], BF16,
                        kind="ExternalInput").ap()
    w8 = nc.dram_tensor("w8", [D_IN, NCH * N_CAPS * 128], BF16,
                        kind="ExternalInput").ap()
    wd = nc.dram_tensor("wd", [128, NCH * N_CAPS * 16], BF16,
                        kind="ExternalInput").ap()
    out = nc.dram_tensor("out", [POS, N_CAPS * 16], F32,
                         kind="ExternalOutput").ap()

    with tile.TileContext(nc) as tc:
        for _rep in range(repeat):
            with tc.tile_pool(name="const", bufs=1) as const, \
                 tc.tile_pool(name="work", bufs=3) as work:
                # per-chunk DMAs, g-ordered, so matmul g can start as soon as
                # its chunk lands. W is sent compact (w8, the 8x-smaller
                # nonzero block of the block-diagonal layout); x is sent with
                # e on partitions so every K=16 matmul sits at base 0.
                # one DMA per tensor (HWDGE costs ~630ns/descriptor), the two
                # s0-path tensors and the two u_hat tensors on separate queues
                w8_t = const.tile([D_IN, NCH * N_CAPS * 128], BF16)
                xtb_t = const.tile([D_IN, N_IN * POS], BF16)
                xf_t = const.tile([128, NCH * POS], BF16)
                wd_t = const.tile([128, NCH * N_CAPS * 16], BF16)
                nc.scalar.dma_start(wd_t[:], wd[:])
                nc.sync.dma_start(xtb_t[:], xT[:])
                nc.scalar.dma_start(xf_t[:], xF[:])
                nc.sync.dma_start(w8_t[:], w8[:])
                out_t = const.tile([BLK, NBLK * 160], F32)

                def split(lo_hi_op):
                    """Issue op on DVE for f<FD slice and Pool for the rest."""
                    lo_hi_op(nc.vector, 0, FD)
                    lo_hi_op(nc.gpsimd, FD, NF)

                with tc.tile_pool(name="ubp", bufs=2) as ubp, \
                     tc.tile_pool(name="psum_s", bufs=2, space="PSUM") as psum_s, \
                     tc.tile_pool(name="big", bufs=1) as big, \
                     tc.tile_pool(name="psum_u", bufs=2, space="PSUM") as psum_u:

                    def uhat_block(b):
                        """u_hat (PSUM, K=16 bf16 matmuls vs compact W) and
                        s0 (dense K=128 chain), drained per g-chunk."""
                        ub = ubp.tile([BLK, NF * 2048], BF16, tag="ub")
                        s0_t = work.tile([BLK, 160], F32, tag="s0")
                        ps = psum_s.tile([BLK, 160], F32, tag="ps")
                        ub5 = ub[:].rearrange("p (f g i c) -> p f g i c",
                                              f=NF, g=NCH, i=8)
                        for g in range(NCH):
                            up3 = psum_u.tile([BLK, NF * 256], F32, tag="up3")
                            nc.tensor.matmul(
                                ps[:],
                                xf_t[:, g * POS + b * BLK:
                                     g * POS + (b + 1) * BLK],
                                wd_t[:, g * 160:(g + 1) * 160],
                                start=(g == 0), stop=(g == NCH - 1))
                            for i8 in range(8):
                                lhs = xtb_t[:, (g * 8 + i8) * POS + b * BLK:
                                            (g * 8 + i8) * POS + (b + 1) * BLK]
                                rhs = w8_t[:, (g * 8 + i8) * 160:
                                           (g * 8 + i8 + 1) * 160]
                                # a matmul PSUM write must not cross a 2KB
                                # bank boundary: split i8=3 (@1920B) / 6
                                # (@3840B) regions at the boundary.
                                cut = {3: 32, 6: 64}.get(i8)
                                if cut is None:
                                    nc.tensor.matmul(
                                        up3[:, i8 * 160:(i8 + 1) * 160],
                                        lhs, rhs, start=True, stop=True)
                                else:
                                    nc.tensor.matmul(
                                        up3[:, i8 * 160:i8 * 160 + cut],
                                        lhs, rhs[:, 0:cut],
                                        start=True, stop=True)
                                    nc.tensor.matmul(
                                        up3[:, i8 * 160 + cut:(i8 + 1) * 160],
                                        lhs, rhs[:, cut:160],
                                        start=True, stop=True)
                            nc.scalar.activation(
                                ub5[:, :, g],
                                up3[:].rearrange("p (i f c) -> p f i c",
                                                 i=8, c=32),
                                mybir.ActivationFunctionType.Copy)
                        nc.scalar.activation(
                            s0_t[:], ps[:],
                            mybir.ActivationFunctionType.Copy,
                            scale=1.0 / N_IN)
                        # v0 = squash(s0), straight into bf16 vb16
                        A = work.tile([BLK, NF * 128], BF16, tag="A")
                        vb16 = work.tile([BLK, 160], BF16, tag="vb16")
                        _squash_on(nc, work, s0_t,
                                   vb16[:].rearrange("p (f o n) -> p f o n",
                                                     o=16, n=2), lowp=True)
                        return dict(ub=ub, s0_t=s0_t, A=A, vb16=vb16)

                    def route_iter(b, it, st):
                        """One routing iteration for block b (agreement then
                        weighted v-sum + squash)."""
                        ub, s0_t, A, vb16 = st["ub"], st["s0_t"], st["A"], st["vb16"]
                        Ub = ub[:].rearrange("p (f gi on) -> p f gi on",
                                             f=NF, gi=64, on=32)
                        # agreement: A (+)= sum_o U*v  (tree over o=16).
                        # All ops use (f, gi, on-k) 4D views, sliceable by f
                        # (engine split) AND gi (block 0 it0 runs in g-halves
                        # so work starts after only half the u_hat drains).
                        P = big.tile([BLK, NF * 2048], BF16, tag="PQ")
                        Pv = P[:].rearrange("p (f gi on) -> p f gi on",
                                            f=NF, gi=64, on=32)
                        vbb = vb16[:].rearrange("p (f on) -> p f on", on=32) \
                            .unsqueeze(2).broadcast_to([BLK, NF, 64, 32])
                        t1 = big.tile([BLK, NF * 1024], BF16, tag="tq1")
                        t1v = t1[:].rearrange("p (f gi on) -> p f gi on",
                                              f=NF, gi=64, on=16)
                        t2 = big.tile([BLK, NF * 512], BF16, tag="tq2")
                        t2v = t2[:].rearrange("p (f gi on) -> p f gi on",
                                              f=NF, gi=64, on=8)
                        t3 = big.tile([BLK, NF * 256], BF16, tag="tq3")
                        t3v = t3[:].rearrange("p (f gi on) -> p f gi on",
                                              f=NF, gi=64, on=4)
                        Av = A[:].rearrange("p (f gi n) -> p f gi n",
                                            f=NF, gi=64)

                        def p_stage(tail, gs, ge):
                            split(lambda e, lo, hi: e.tensor_mul(
                                Pv[:, lo:hi, gs:ge], Ub[:, lo:hi, gs:ge],
                                vbb[:, lo:hi, gs:ge]))
                            with nc.allow_low_precision("bf16 tree sums"):
                                split(lambda e, lo, hi: e.tensor_add(
                                    t1v[:, lo:hi, gs:ge],
                                    Pv[:, lo:hi, gs:ge, 0:16],
                                    Pv[:, lo:hi, gs:ge, 16:32]))
                                split(lambda e, lo, hi: e.tensor_add(
                                    t2v[:, lo:hi, gs:ge],
                                    t1v[:, lo:hi, gs:ge, 0:8],
                                    t1v[:, lo:hi, gs:ge, 8:16]))
                                split(lambda e, lo, hi: e.tensor_add(
                                    t3v[:, lo:hi, gs:ge],
                                    t2v[:, lo:hi, gs:ge, 0:4],
                                    t2v[:, lo:hi, gs:ge, 4:8]))
                                split(lambda e, lo, hi: e.tensor_add(
                                    tail[:, lo:hi, gs:ge],
                                    t3v[:, lo:hi, gs:ge, 0:2],
                                    t3v[:, lo:hi, gs:ge, 2:4]))

                        with nc.allow_low_precision("bf16 tree sums"):
                            if it == 0:
                                if b == 0:
                                    p_stage(Av, 0, 32)
                                    p_stage(Av, 32, 64)
                                else:
                                    p_stage(Av, 0, 64)
                            else:
                                agr = work.tile([BLK, NF * 128], BF16,
                                                tag="agr")
                                agv = agr[:].rearrange(
                                    "p (f gi n) -> p f gi n", f=NF, gi=64)
                                p_stage(agv, 0, 64)
                                Af = A[:].rearrange("p (f gn) -> p f gn",
                                                    gn=128)
                                agf = agr[:].rearrange("p (f gn) -> p f gn",
                                                       gn=128)
                                split(lambda e, lo, hi: e.tensor_add(
                                    Af[:, lo:hi], Af[:, lo:hi],
                                    agf[:, lo:hi]))
                        # v-sum: s = s0 + sum_gi A*U  (tree over gi=64)
                        # op dims (s=(f,gi), o, n2): broadcast of A over o
                        # sits mid-AP, n2 stays contiguous-last -> 2x.
                        Q = big.tile([BLK, NF * 2048], BF16, tag="PQ")
                        Qv = Q[:].rearrange("p (s o n) -> p s o n",
                                            o=16, n=2)
                        Uq = ub[:].rearrange("p (s o n) -> p s o n",
                                             o=16, n=2)
                        Ab = A[:].rearrange("p (s n) -> p s n", n=2) \
                            .unsqueeze(2).broadcast_to([BLK, NF * 64, 16, 2])
                        split(lambda e, lo, hi: e.tensor_mul(
                            Qv[:, lo * 64:hi * 64], Uq[:, lo * 64:hi * 64],
                            Ab[:, lo * 64:hi * 64]))
                        with nc.allow_low_precision("bf16 tree sums"):
                            Qg = Q[:].rearrange("p (f g s) -> p f g s",
                                                f=NF, g=64, s=32)
                            q1 = big.tile([BLK, NF * 1024], BF16, tag="tq1")
                            q1v = q1[:].rearrange("p (f g s) -> p f g s",
                                                  f=NF, g=32, s=32)
                            split(lambda e, lo, hi: e.tensor_add(
                                q1v[:, lo:hi], Qg[:, lo:hi, 0:32],
                                Qg[:, lo:hi, 32:64]))
                            q2 = big.tile([BLK, NF * 512], BF16, tag="tq2")
                            q2v = q2[:].rearrange("p (f g s) -> p f g s",
                                                  f=NF, g=16, s=32)
                            split(lambda e, lo, hi: e.tensor_add(
                                q2v[:, lo:hi], q1v[:, lo:hi, 0:16],
                                q1v[:, lo:hi, 16:32]))
                            q3 = big.tile([BLK, NF * 256], BF16, tag="tq3")
                            q3v = q3[:].rearrange("p (f g s) -> p f g s",
                                                  f=NF, g=8, s=32)
                            split(lambda e, lo, hi: e.tensor_add(
                                q3v[:, lo:hi], q2v[:, lo:hi, 0:8],
                                q2v[:, lo:hi, 8:16]))
                            q4 = big.tile([BLK, NF * 128], BF16, tag="q4")
                            q4v = q4[:].rearrange("p (f g s) -> p f g s",
                                                  f=NF, g=4, s=32)
                            split(lambda e, lo, hi: e.tensor_add(
                                q4v[:, lo:hi], q3v[:, lo:hi, 0:4],
                                q3v[:, lo:hi, 4:8]))
                            q5 = big.tile([BLK, NF * 64], BF16, tag="q5")
                            q5v = q5[:].rearrange("p (f g s) -> p f g s",
                                                  f=NF, g=2, s=32)
                            split(lambda e, lo, hi: e.tensor_add(
                                q5v[:, lo:hi], q4v[:, lo:hi, 0:2],
                                q4v[:, lo:hi, 2:4]))
                            s_blk = work.tile([BLK, 160], F32, tag="s_blk")
                            sv = s_blk[:].rearrange("p (f g s) -> p f g s",
                                                    f=NF, g=1, s=32)
                            split(lambda e, lo, hi: e.tensor_add(
                                sv[:, lo:hi], q5v[:, lo:hi, 0:1],
                                q5v[:, lo:hi, 1:2]))
                        # + s0 (same (f, o, n2) layout)
                        nc.vector.tensor_add(s_blk[:], s_blk[:], s0_t[:])
                        # squash (batched over the block's 10 capsules)
                        if it == 0:
                            _squash_on(nc, work, s_blk,
                                       vb16[:].rearrange(
                                           "p (f o n) -> p f o n",
                                           o=16, n=2), lowp=True)
                        else:
                            dst = out_t[:, b * 160:(b + 1) * 160]
                            _squash_on(nc, work, s_blk,
                                       dst.rearrange(
                                           "p (f n o) -> p f o n",
                                           n=2, o=16))
                            nc.sync.dma_start(
                                out[b * BLK:(b + 1) * BLK, :],
                                out_t[:, b * 160:(b + 1) * 160])

                    for b in range(NBLK):
                        st = uhat_block(b)
                        route_iter(b, 0, st)
                        route_iter(b, 1, st)
    nc.compile()
    return nc


def _host_prep(inputs, W):
    """Build per-core input maps from full inputs."""
    import ml_dtypes
    x = np.ascontiguousarray(inputs, dtype=np.float32).reshape(B, R * C, IE)
    Wf = np.ascontiguousarray(W, dtype=np.float32)  # [n, i, e, o]
    # w8[e, (g, i8, f, o, n2)] - compact nonzero block of BD(W)
    w8 = Wf.reshape(NF, 2, NCH, 8, D_IN, CAPS_DIM)  # [f, n2, g, i8, e, o]
    w8 = w8.transpose(4, 2, 3, 0, 5, 1)             # [e, g, i8, f, o, n2]
    w8_a = np.ascontiguousarray(w8).reshape(
        D_IN, NCH * N_CAPS * 128).astype(ml_dtypes.bfloat16)
    # wd[(i8,e), (g, f, o, n2)] - dense W for the s0 chain
    wd = Wf.reshape(NF, 2, N_IN, D_IN, CAPS_DIM)     # [f, n2, i, e, o]
    wd = wd.transpose(2, 3, 0, 4, 1)                 # [i, e, f, o, n2]
    wd = wd.reshape(NCH, 128, N_CAPS * 16)           # [g, (i8 e), 160]
    wd = wd.transpose(1, 0, 2).reshape(128, NCH * N_CAPS * 16)
    wd_a = np.ascontiguousarray(wd).astype(ml_dtypes.bfloat16)
    bpc = B // N_CORES
    in_maps = []
    for c in range(N_CORES):
        xc = x[c * bpc:(c + 1) * bpc].reshape(POS, IE)
        # xT[e, (g, i8, pos)] in bf16
        xt = xc.reshape(POS, N_IN, D_IN).transpose(2, 1, 0)
        xt = np.ascontiguousarray(xt).reshape(D_IN, N_IN * POS)
        # xF[(i8, e), (g, pos)] in bf16 - dense-layout x for the s0 chain
        xf = xc.T.reshape(NCH, 128, POS).transpose(1, 0, 2)
        xf = np.ascontiguousarray(xf).reshape(128, NCH * POS)
        in_maps.append({
            "xT": xt.astype(ml_dtypes.bfloat16),
            "xF": xf.astype(ml_dtypes.bfloat16),
            "w8": w8_a,
            "wd": wd_a,
        })
    return in_maps


_NC_CACHE = []


def kernel(inputs: np.ndarray, W: np.ndarray) -> np.ndarray:
    in_maps = _host_prep(inputs, W)
    if not _NC_CACHE:
        _NC_CACHE.append(build_kernel())
    nc = _NC_CACHE[0]
    res = run_bass_kernel_spmd(nc, in_maps, list(range(N_CORES)))
    outs = [res.results[c]["out"] for c in range(N_CORES)]
    full = np.concatenate(outs, axis=0)  # [3136, 160]
    return full.reshape(B, R, C, N_CAPS, CAPS_DIM)


# revision 46
# speedup vs baseline: 1.3385x; 1.3385x over previous
"""CapsLayer2D Trainium2 kernel (8-core SPMD, data-parallel over batch).

Math: per position p (of B*R*C) and capsule n:
  U[n,i,o] = sum_e x[p,i,e] * W[n,i,e,o]          (u_hat)
  b0 = 1/64; 2x { v = squash(sum_i b*U); b += sum_o U*v }; out = squash(sum_i b*U)

Since b = 1/64 + A (A = accumulated agreement), sum_i b*U = s0 + sum_i A*U
with s0 = (1/64) sum_i U - so routing tracks A only (no memset, A in bf16).

Mapping:
  - 8 cores, 2 batches each -> 392 positions/core, 4 pos-blocks of 98.
  - u_hat per (block, g-chunk): one K=16 matmul per input capsule (g,i8)
    against the compact nonzero block w8 of the block-diagonal W (x is sent
    with e on partitions so every matmul sits at base partition 0); PSUM
    [98,1280] per g, Act-drained to bf16 SBUF. Writes that would cross a
    2KB PSUM bank are split (HW corrupts bank-crossing matmul stores).
    s0 comes from a dense K=128 accumulating chain (8 matmuls/block).
  - Routing iterations as elementwise mul + segmented-reduce add-trees.
  - u_hat columns are laid out (g, i8, o, n2) [NOT (..., n2, o)]: with the
    capsule-pair index n2 innermost, every operand of every big mul/tree is
    last-dim-contiguous bf16 (broadcasts over v / A land on middle dims), so
    all DVE TensorTensor ops hit the 2x perf mode.
  - The mul/tree work of capsule-pair unit f=4 runs on the Pool engine
    (gpsimd) in parallel with f=0..3 on DVE, roughly balancing the two.
  - Host pre-builds bf16 xT (e-major), xF (dense-major), w8, wd.
"""
import numpy as np

import concourse.bacc as bacc
import concourse.bass as bass
import concourse.mybir as mybir
import concourse.tile as tile
from concourse.bass_utils import run_bass_kernel_spmd

N_CORES = 8
B, R, C = 16, 14, 14
N_IN, D_IN = 64, 16          # i, e
N_CAPS, CAPS_DIM = 10, 16    # n, o
IE = N_IN * D_IN             # 1024
POS = (B // N_CORES) * R * C # 392 positions per core
BLK = 98                     # pos-block size
NBLK = POS // BLK            # 4
NF = N_CAPS // 2             # 5 units of 2 capsules
NCH = IE // 128              # 8 contraction chunks
F32 = mybir.dt.float32

# u_hat matmuls run in bf16 (1 col/cycle at any N; fp32 is 4x slower,
# fp32r needs producer-side rounding the DMA can't provide).
BF16 = mybir.dt.bfloat16

# DVE handles capsule-pair units f < FD, Pool (gpsimd) handles f >= FD.
FD = 4


def _squash_on(nc, pool, s_t, v_ap, lowp=False):
    """v = squash(s) for an (f, o, n2)-ordered s tile [P, 160] (routing
    layout). v_ap must be a [P, NF, 16, 2]-shaped AP in op dims (f, o, n2).
    Square runs on DVE (cheap f32 mul) to avoid an Act round-trip; only
    Sqrt uses the Act engine.
    """
    P = s_t.shape[0]
    s_fon = s_t[:].rearrange("p (f o n) -> p f o n", o=16, n=2)
    sq = pool.tile([P, 160], F32, tag="sq")
    nc.vector.tensor_mul(sq[:], s_t[:], s_t[:])
    # reduce over o (middle in storage): 4D view [P, f, n2, o], axis=X
    q = pool.tile([P, N_CAPS], F32, tag="q")
    nc.vector.tensor_reduce(q[:].rearrange("p (f n) -> p f n", n=2),
                            sq[:].rearrange("p (f o n) -> p f n o",
                                            o=16, n=2),
                            axis=mybir.AxisListType.X, op=mybir.AluOpType.add)
    rt = pool.tile([P, N_CAPS], F32, tag="rt")
    nc.scalar.activation(rt[:], q[:], mybir.ActivationFunctionType.Sqrt)
    qp = pool.tile([P, N_CAPS], F32, tag="qp")
    nc.vector.tensor_scalar_add(qp[:], q[:], 1.0)
    rc = pool.tile([P, N_CAPS], F32, tag="rc")
    nc.vector.reciprocal(rc[:], qp[:])
    al = pool.tile([P, N_CAPS], F32, tag="al")
    nc.vector.tensor_mul(al[:], rt[:], rc[:])
    # al is [P, (f, n2)]; broadcast over o (middle dim of (f, o, n2))
    alb = al[:].rearrange("p (f n) -> p f n", n=2) \
        .unsqueeze(2).broadcast_to([P, NF, 16, 2])
    if lowp:
        with nc.allow_low_precision("bf16 v"):
            nc.vector.tensor_mul(v_ap, s_fon, alb)
    else:
        nc.vector.tensor_mul(v_ap, s_fon, alb)


def build_kernel(dbg=False, repeat=1):
    nc = bacc.Bacc("TRN2", target_bir_lowering=False, debug=False,
                   num_devices=N_CORES)
    xT = nc.dram_tensor("xT", [D_IN, N_IN * POS], BF16,
                        kind="ExternalInput").ap()
    xF = nc.dram_tensor("xF", [128, NCH * POS], BF16,
                        kind="ExternalInput").ap()
    w8 = nc.dram_tensor("w8", [D_IN, NCH * N_CAPS * 128], BF16,
                        kind="ExternalInput").ap()
    wd = nc.dram_tensor("wd", [128, NCH * N_CAPS * 16], BF16,
                        kind="ExternalInput").ap()
    out = nc.dram_tensor("out", [POS, N_CAPS * 16], F32,
                         kind="ExternalOutput").ap()

    with tile.TileContext(nc) as tc:
        for _rep in range(repeat):
            with tc.tile_pool(name="const", bufs=1) as const, \
                 tc.tile_pool(name="work", bufs=3) as work:
                # per-chunk DMAs, g-ordered, so matmul g can start as soon as
                # its chunk lands. W is sent compact (w8, the 8x-smaller
                # nonzero block of the block-diagonal layout); x is sent with
                # e on partitions so every K=16 matmul sits at base 0.
                # one DMA per tensor (HWDGE costs ~630ns/descriptor), the two
                # s0-path tensors and the two u_hat tensors on separate queues
                w8_t = const.tile([D_IN, NCH * N_CAPS * 128], BF16)
                xtb_t = const.tile([D_IN, N_IN * POS], BF16)
                xf_t = const.tile([128, NCH * POS], BF16)
                wd_t = const.tile([128, NCH * N_CAPS * 16], BF16)
                nc.scalar.dma_start(wd_t[:], wd[:])
                nc.sync.dma_start(xtb_t[:], xT[:])
                nc.scalar.dma_start(xf_t[:], xF[:])
                nc.sync.dma_start(w8_t[:], w8[:])
                out_t = const.tile([BLK, NBLK * 160], F32)

                def split(lo_hi_op):
                    """Issue op on DVE for f<FD slice and Pool for the rest."""
                    lo_hi_op(nc.vector, 0, FD)
                    lo_hi_op(nc.gpsimd, FD, NF)

                with tc.tile_pool(name="ubp", bufs=2) as ubp, \
                     tc.tile_pool(name="psum_s", bufs=2, space="PSUM") as psum_s, \
                     tc.tile_pool(name="big", bufs=1) as big, \
                     tc.tile_pool(name="psum_u", bufs=2, space="PSUM") as psum_u:

                    def uhat_block(b):
                        """u_hat (PSUM, K=16 bf16 matmuls vs compact W) and
                        s0 (dense K=128 chain), drained per g-chunk."""
                        ub = ubp.tile([BLK, NF * 2048], BF16, tag="ub")
                        s0_t = work.tile([BLK, 160], F32, tag="s0")
                        ps = psum_s.tile([BLK, 160], F32, tag="ps")
                        ub5 = ub[:].rearrange("p (f g i c) -> p f g i c",
                                              f=NF, g=NCH, i=8)
                        for g in range(NCH):
                            up3 = psum_u.tile([BLK, NF * 256], F32, tag="up3")
                            nc.tensor.matmul(
                                ps[:],
                                xf_t[:, g * POS + b * BLK:
                                     g * POS + (b + 1) * BLK],
                                wd_t[:, g * 160:(g + 1) * 160],
                                start=(g == 0), stop=(g == NCH - 1))
                            for i8 in range(8):
                                lhs = xtb_t[:, (g * 8 + i8) * POS + b * BLK:
                                            (g * 8 + i8) * POS + (b + 1) * BLK]
                                rhs = w8_t[:, (g * 8 + i8) * 160:
                                           (g * 8 + i8 + 1) * 160]
                                # a matmul PSUM write must not cross a 2KB
                                # bank boundary: split i8=3 (@1920B) / 6
                                # (@3840B) regions at the boundary.
                                cut = {3: 32, 6: 64}.get(i8)
                                if cut is None:
                                    nc.tensor.matmul(
                                        up3[:, i8 * 160:(i8 + 1) * 160],
                                        lhs, rhs, start=True, stop=True)
                                else:
                                    nc.tensor.matmul(
                                        up3[:, i8 * 160:i8 * 160 + cut],
                                        lhs, rhs[:, 0:cut],
                                        start=True, stop=True)
                                    nc.tensor.matmul(
                                        up3[:, i8 * 160 + cut:(i8 + 1) * 160],
                                        lhs, rhs[:, cut:160],
                                        start=True, stop=True)
                            nc.scalar.activation(
                                ub5[:, :, g],
                                up3[:].rearrange("p (i f c) -> p f i c",
                                                 i=8, c=32),
                                mybir.ActivationFunctionType.Copy)
                        nc.scalar.activation(
                            s0_t[:], ps[:],
                            mybir.ActivationFunctionType.Copy,
                            scale=1.0 / N_IN)
                        # v0 = squash(s0), straight into bf16 vb16
                        A = work.tile([BLK, NF * 128], BF16, tag="A")
                        vb16 = work.tile([BLK, 160], BF16, tag="vb16")
                        _squash_on(nc, work, s0_t,
                                   vb16[:].rearrange("p (f o n) -> p f o n",
                                                     o=16, n=2), lowp=True)
                        return dict(ub=ub, s0_t=s0_t, A=A, vb16=vb16)

                    def route_iter(b, it, st):
                        """One routing iteration for block b (agreement then
                        weighted v-sum + squash)."""
                        ub, s0_t, A, vb16 = st["ub"], st["s0_t"], st["A"], st["vb16"]
                        Ub = ub[:].rearrange("p (f gi on) -> p f gi on",
                                             f=NF, gi=64, on=32)
                        # agreement: A (+)= sum_o U*v  (tree over o=16).
                        # All ops use (f, gi, on-k) 4D views, sliceable by f
                        # (engine split) AND gi (block 0 it0 runs in g-halves
                        # so work starts after only half the u_hat drains).
                        P = big.tile([BLK, NF * 2048], BF16, tag="PQ")
                        Pv = P[:].rearrange("p (f gi on) -> p f gi on",
                                            f=NF, gi=64, on=32)
                        vbb = vb16[:].rearrange("p (f on) -> p f on", on=32) \
                            .unsqueeze(2).broadcast_to([BLK, NF, 64, 32])
                        t1 = big.tile([BLK, NF * 1024], BF16, tag="tq1")
                        t1v = t1[:].rearrange("p (f gi on) -> p f gi on",
                                              f=NF, gi=64, on=16)
                        t2 = big.tile([BLK, NF * 512], BF16, tag="tq2")
                        t2v = t2[:].rearrange("p (f gi on) -> p f gi on",
                                              f=NF, gi=64, on=8)
                        t3 = big.tile([BLK, NF * 256], BF16, tag="tq3")
                        t3v = t3[:].rearrange("p (f gi on) -> p f gi on",
                                              f=NF, gi=64, on=4)
                        Av = A[:].rearrange("p (f gi n) -> p f gi n",
                                            f=NF, gi=64)

                        def p_stage(tail, gs, ge):
                            split(lambda e, lo, hi: e.tensor_mul(
                                Pv[:, lo:hi, gs:ge], Ub[:, lo:hi, gs:ge],
                                vbb[:, lo:hi, gs:ge]))
                            with nc.allow_low_precision("bf16 tree sums"):
                                split(lambda e, lo, hi: e.tensor_add(
                                    t1v[:, lo:hi, gs:ge],
                                    Pv[:, lo:hi, gs:ge, 0:16],
                                    Pv[:, lo:hi, gs:ge, 16:32]))
                                split(lambda e, lo, hi: e.tensor_add(
                                    t2v[:, lo:hi, gs:ge],
                                    t1v[:, lo:hi, gs:ge, 0:8],
                                    t1v[:, lo:hi, gs:ge, 8:16]))
                                split(lambda e, lo, hi: e.tensor_add(
                                    t3v[:, lo:hi, gs:ge],
                                    t2v[:, lo:hi, gs:ge, 0:4],
                                    t2v[:, lo:hi, gs:ge, 4:8]))
                                split(lambda e, lo, hi: e.tensor_add(
                                    tail[:, lo:hi, gs:ge],
                                    t3v[:, lo:hi, gs:ge, 0:2],
                                    t3v[:, lo:hi, gs:ge, 2:4]))

                        with nc.allow_low_precision("bf16 tree sums"):
                            if it == 0:
                                if b == 0:
                                    p_stage(Av, 0, 32)
                                    p_stage(Av, 32, 64)
                                else:
                                    p_stage(Av, 0, 64)
                            else:
                                agr = work.tile([BLK, NF * 128], BF16,
                                                tag="agr")
                                agv = agr[:].rearrange(
                                    "p (f gi n) -> p f gi n", f=NF, gi=64)
                                p_stage(agv, 0, 64)
                                Af = A[:].rearrange("p (f gn) -> p f gn",
                                                    gn=128)
                                agf = agr[:].rearrange("p (f gn) -> p f gn",
                                                       gn=128)
                                split(lambda e, lo, hi: e.tensor_add(
                                    Af[:, lo:hi], Af[:, lo:hi],
                                    agf[:, lo:hi]))
                        # v-sum: s = s0 + sum_gi A*U  (tree over gi=64)
                        # op dims (s=(f,gi), o, n2): broadcast of A over o
                        # sits mid-AP, n2 stays contiguous-last -> 2x.
                        Q = big.tile([BLK, NF * 2048], BF16, tag="PQ")
                        Qv = Q[:].rearrange("p (s o n) -> p s o n",
                                            o=16, n=2)
                        Uq = ub[:].rearrange("p (s o n) -> p s o n",
                                             o=16, n=2)
                        Ab = A[:].rearrange("p (s n) -> p s n", n=2) \
                            .unsqueeze(2).broadcast_to([BLK, NF * 64, 16, 2])
                        split(lambda e, lo, hi: e.tensor_mul(
                            Qv[:, lo * 64:hi * 64], Uq[:, lo * 64:hi * 64],
                            Ab[:, lo * 64:hi * 64]))
                        with nc.allow_low_precision("bf16 tree sums"):
                            Qg = Q[:].rearrange("p (f g s) -> p f g s",
                                                f=NF, g=64, s=32)
                            q1 = big.tile([BLK, NF * 1024], BF16, tag="tq1")
                            q1v = q1[:].rearrange("p (f g s) -> p f g s",
                                                  f=NF, g=32, s=32)
                            split(lambda e, lo, hi: e.tensor_add(
                                q1v[:, lo:hi], Qg[:, lo:hi, 0:32],
                                Qg[:, lo:hi, 32:64]))
                            q2 = big.tile([BLK, NF * 512], BF16, tag="tq2")
                            q2v = q2[:].rearrange("p (f g s) -> p f g s",
                                                  f=NF, g=16, s=32)
                            split(lambda e, lo, hi: e.tensor_add(
                                q2v[:, lo:hi], q1v[:, lo:hi, 0:16],
                                q1v[:, lo:hi, 16:32]))
                            q3 = big.tile([BLK, NF * 256], BF16, tag="tq3")
                            q3v = q3[:].rearrange("p (f g s) -> p f g s",
                                                  f=NF, g=8, s=32)
                            split(lambda e, lo, hi: e.tensor_add(
                                q3v[:, lo:hi], q2v[:, lo:hi, 0:8],
                                q2v[:, lo:hi, 8:16]))
                            q4 = big.tile([BLK, NF * 128], BF16, tag="q4")
                            q4v = q4[:].rearrange("p (f g s) -> p f g s",
                                                  f=NF, g=4, s=32)
                            split(lambda e, lo, hi: e.tensor_add(
                                q4v[:, lo:hi], q3v[:, lo:hi, 0:4],
                                q3v[:, lo:hi, 4:8]))
                            q5 = big.tile([BLK, NF * 64], BF16, tag="q5")
                            q5v = q5[:].rearrange("p (f g s) -> p f g s",
                                                  f=NF, g=2, s=32)
                            split(lambda e, lo, hi: e.tensor_add(
                                q5v[:, lo:hi], q4v[:, lo:hi, 0:2],
                                q4v[:, lo:hi, 2:4]))
                            s_blk = work.tile([BLK, 160], F32, tag="s_blk")
                            sv = s_blk[:].rearrange("p (f g s) -> p f g s",
                                                    f=NF, g=1, s=32)
                            split(lambda e, lo, hi: e.tensor_add(
                                sv[:, lo:hi], q5v[:, lo:hi, 0:1],
                                q5v[:, lo:hi, 1:2]))
                        # + s0 (same (f, o, n2) layout)
                        nc.vector.tensor_add(s_blk[:], s_blk[:], s0_t[:])
                        # squash (batched over the block's 10 capsules)
                        if it == 0:
                            _squash_on(nc, work, s_blk,
                                       vb16[:].rearrange(
                                           "p (f o n) -> p f o n",
                                           o=16, n=2), lowp=True)
                        else:
                            dst = out_t[:, b * 160:(b + 1) * 160]
                            _squash_on(nc, work, s_blk,
                                       dst.rearrange(
                                           "p (f n o) -> p f o n",
                                           n=2, o=16))
                            nc.sync.dma_start(
                                out[b * BLK:(b + 1) * BLK, :],
                                out_t[:, b * 160:(b + 1) * 160])

                    for b in range(NBLK):
                        st = uhat_block(b)
                        route_iter(b, 0, st)
                        route_iter(b, 1, st)
    nc.compile()
    return nc


def _host_prep(inputs, W):
    """Build per-core input maps from full inputs."""
    import ml_dtypes
    x = np.ascontiguousarray(inputs, dtype=np.float32).reshape(B, R * C, IE)
    Wf = np.ascontiguousarray(W, dtype=np.float32)  # [n, i, e, o]
    # w8[e, (g, i8, f, o, n2)] - compact nonzero block of BD(W)
    w8 = Wf.reshape(NF, 2, NCH, 8, D_IN, CAPS_DIM)  # [f, n2, g, i8, e, o]
    w8 = w8.transpose(4, 2, 3, 0, 5, 1)             # [e, g, i8, f, o, n2]
    w8_a = np.ascontiguousarray(w8).reshape(
        D_IN, NCH * N_CAPS * 128).astype(ml_dtypes.bfloat16)
    # wd[(i8,e), (g, f, o, n2)] - dense W for the s0 chain
    wd = Wf.reshape(NF, 2, N_IN, D_IN, CAPS_DIM)     # [f, n2, i, e, o]
    wd = wd.transpose(2, 3, 0, 4, 1)                 # [i, e, f, o, n2]
    wd = wd.reshape(NCH, 128, N_CAPS * 16)           # [g, (i8 e), 160]
    wd = wd.transpose(1, 0, 2).reshape(128, NCH * N_CAPS * 16)
    wd_a = np.ascontiguousarray(wd).astype(ml_dtypes.bfloat16)
    bpc = B // N_CORES
    in_maps = []
    for c in range(N_CORES):
        xc = x[c * bpc:(c + 1) * bpc].reshape(POS, IE)
        # xT[e, (g, i8, pos)] in bf16
        xt = xc.reshape(POS, N_IN, D_IN).transpose(2, 1, 0)
        xt = np.ascontiguousarray(xt).reshape(D_IN, N_IN * POS)
        # xF[(i8, e), (g, pos)] in bf16 - dense-layout x for the s0 chain
        xf = xc.T.reshape(NCH, 128, POS).transpose(1, 0, 2)
        xf = np.ascontiguousarray(xf).reshape(128, NCH * POS)
        in_maps.append({
            "xT": xt.astype(ml_dtypes.bfloat16),
            "xF": xf.astype(ml_dtypes.bfloat16),
            "w8": w8_a,
            "wd": wd_a,
        })
    return in_maps


_NC_CACHE = []


def kernel(inputs: np.ndarray, W: np.ndarray) -> np.ndarray:
    in_maps = _host_prep(inputs, W)
    if not _NC_CACHE:
        _NC_CACHE.append(build_kernel())
    nc = _NC_CACHE[0]
    res = run_bass_kernel_spmd(nc, in_maps, list(range(N_CORES)))
    outs = [res.results[c]["out"] for c in range(N_CORES)]
    full = np.concatenate(outs, axis=0)  # [3136, 160]
    return full.reshape(B, R, C, N_CAPS, CAPS_DIM)
